# revision 62
# baseline (speedup 1.0000x reference)
"""CBTree (levelized complete 4-ary tree, depth 8, D=512) forward on 8 TRN2 NeuronCores.

Math: per level,  h = tanh(x + sum_b lc[b]*(h_b @ wl.T) + rc[b]*(h_b @ wr.T)).
By linearity the per-child matmuls collapse to two matmuls of weighted child
sums:  agg = u_l' @ (wl.T/3) + u_r' @ (wr.T/3)  with integer-coefficient sums
u_l' = 3 h0 + 2 h1 + h2 and u_r' = h1 + 2 h2 + 3 h3  (4x FLOP reduction).

Sharding: the 16 level-2 subtrees are sharded 2-per-core; each core runs
global levels 7..2 of its forest locally, one AllGather collects the 16
level-2 hiddens, and every core redundantly computes the tiny levels 1+0.

Layout: feature-major ([128 partitions, 4 d-tiles, n]); node storage at every
level uses a *localized* child-block grouping: level-(l+1) columns are stored
in groups of 4*W (W = the compute-chunk width of level l), each group holding
the four child blocks of one parent chunk.  This makes every level-l compute
sub-chunk depend on exactly one contiguous, already-finished range of the
level-(l+1) storage, so levels pipeline into each other with no drain.

Engines: leaves and the big levels' x ride fp8e3m4 (halves the dominant DMA
stream; ~1.4e-2 end-to-end rel err vs the 2e-2 gate).  Each leaf block
crosses to f16 once, the four 1x-rate conversions split across Act (2,
scaled copies), Pool (1) and DVE (1); DVE then builds Ul/Ur in f16 4x/2x
modes.  PE does the K=1024 matmul + an identity-matmul x-injection (emitted
last in each PSUM group so matmuls start before x arrives), ScalarE runs
tanh on [128,2,w] PSUM slabs.  The tiny tail levels (w<=32) use the direct
4-child form (stacked Vb weights, no U-build) to shorten the serial
act->mm->act chain; a PE warmup burns the 0.65->2.4GHz p-state ramp during
the DMA fill.  Leaves are one DRAM tensor per core, DMA'd in large chunks.
"""
import sys

import numpy as np

sys.path.insert(0, "/opt/trn_rl_repo")

import concourse.bass as bass  # noqa: E402,F401
import concourse.bacc as bacc  # noqa: E402
import concourse.tile as tile  # noqa: E402
from concourse import mybir  # noqa: E402
from concourse.bass_utils import run_bass_kernel_spmd  # noqa: E402

NCORES = 8
D = 512
NT = 4  # d-tiles of 128
B = 4
DEPTH = 8
# local levels L=0..6 <-> global levels 2..8 ; per-core node counts
NLOC = [2 * 4**l for l in range(7)]  # [2, 8, 32, 128, 512, 2048, 8192]

PRECISION = "f16"  # informational (test.py prints it)

FP32 = mybir.dt.float32
F16 = mybir.dt.float16
F8E3 = mybir.dt.float8e3  # e3m4: 1 cyc/row PE moving operand, half DMA bytes
F16np = np.float16

# L5 leaf-DMA chunk widths (parent columns); compute units split these into
# <=256-col pieces.  Small chunks first so the PE starts early while DMA fills.
L5_CHUNKS = [64, 192, 256, 512, 512, 512]
L5_OFF = [0, 64, 256, 512, 1024, 1536]
# compute-chunk width per level, used for the child-level storage grouping
WGRP = {0: 2, 1: 8, 2: 32, 3: 32, 4: 128}

UNIT = 256  # max compute-unit width


# ---------------------------------------------------------------- host layout
def _child(j: np.ndarray, l: int, b: int) -> np.ndarray:
    """Local node ids at level l -> child b ids at level l+1."""
    r = j >> (2 * l)
    q = j & ((1 << (2 * l)) - 1)
    return (r << (2 * (l + 1))) | (4 * q + b)


def _perms():
    """Storage orders P[l] (local node id per storage column) for l=0..6."""
    P = {0: np.arange(2, dtype=np.int64)}
    for l in range(6):
        par = P[l]
        if l < 5:
            groups = par.reshape(-1, WGRP[l])
        else:
            groups = [par[o:o + w] for o, w in zip(L5_OFF, L5_CHUNKS)]
        out = []
        for g in groups:
            for b in range(B):
                out.append(_child(g, l, b))
        P[l + 1] = np.concatenate(out)
    return P


_P = _perms()


# ---------------------------------------------------------------- device build
def _build_nc(with_tail=True):
    nc = bacc.Bacc(
        "TRN2", target_bir_lowering=False, debug=False, num_devices=NCORES
    )

    # leaves ride fp8e3m4 (~0.4% quant err attenuated through 7 tanh levels):
    # halves the dominant DMA stream.  Each child block is converted to f16
    # exactly once; the four 1x-rate conversions are split across Act (2,
    # as scaled copies), Pool (1) and DVE (1) so no engine exceeds the PE's
    # per-chunk time.
    leaf_d = nc.dram_tensor("leaf", [D, NLOC[6]], F8E3, kind="ExternalInput")
    # x for the two big levels rides fp8e3m4: it is only ever a PE moving
    # operand (identity inject), runs at 1 cyc/row, and its ~0.4% quant error
    # is attenuated through 5+ tanh levels.
    x5_d = nc.dram_tensor("x5", [D, NLOC[5]], F8E3, kind="ExternalInput")
    x4_d = nc.dram_tensor("x4", [D, NLOC[4]], F8E3, kind="ExternalInput")
    # [L3:128 | L2:32 | L1:8 | L0:2 | xt1:4 | xt0:1] = 175 cols
    xsm_d = nc.dram_tensor("xsm", [D, 175], F16, kind="ExternalInput")
    w2_d = nc.dram_tensor("w2", [2 * D, D], F16, kind="ExternalInput")
    # per-child-position matrices Vb = lc_b*wl + rc_b*wr stacked [4D, D]:
    # used by the tiny tail levels (w<=32) to skip the DVE U-build entirely
    w4_d = nc.dram_tensor("w4", [4 * D, D], F16, kind="ExternalInput")
    idm_d = nc.dram_tensor("identm", [128, 128], F16, kind="ExternalInput")
    out_d = nc.dram_tensor("out", [1, D], FP32, kind="ExternalOutput")

    def fm_ap(handle):
        # DRAM [512, n] -> [128p, 4t, n] with feature d = t*128 + p
        return handle.ap().rearrange("(t p) n -> p t n", p=128)

    mult, add = mybir.AluOpType.mult, mybir.AluOpType.add
    Tanh = mybir.ActivationFunctionType.Tanh

    with tile.TileContext(nc) as tc:
        with (
            tc.tile_pool(name="const", bufs=1) as const,
            tc.tile_pool(name="hp", bufs=1) as hp,
            tc.tile_pool(name="leafp", bufs=6) as leafp,
            tc.tile_pool(name="xp", bufs=6) as xp,
            tc.tile_pool(name="up", bufs=6) as up,
            tc.tile_pool(name="tmpp", bufs=2) as tmpp,
            tc.tile_pool(name="psum", bufs=4, space="PSUM") as psum,
            tc.tile_pool(name="dram", bufs=1, space="DRAM") as dram,
        ):
            # ---------------- PE p-state warmup -----------------------------
            # The PE clock ramps 0.65->1.2->2.4 GHz over ~3us of continuous
            # execution.  Burn the ramp on dummy matmuls during the DMA fill
            # so real matmuls start at full clock.
            junk = const.tile([128, 512], F16, tag="junk", name="junk")
            nc.vector.memset(junk[:], 0.0)
            warm_ps = psum.tile([128, 2, 512], FP32, tag="ps", name="warmps")
            for i in range(8):
                nc.tensor.matmul(
                    warm_ps[:, 0, :], junk[:, 0:128], junk[:],
                    start=True, stop=True,
                )

            # ---------------- DMA emission (order = DMA service order) ------
            leaf_sb = {}
            x5_sb = {}

            def dma_leaf(ci):
                w = L5_CHUNKS[ci]
                t = leafp.tile([128, NT, 4 * w], F8E3, tag="leaf", name=f"lb{ci}")
                o = 4 * L5_OFF[ci]
                nc.sync.dma_start(out=t[:], in_=fm_ap(leaf_d)[:, :, o:o + 4 * w])
                leaf_sb[ci] = t

            def dma_x5(ci):
                w = L5_CHUNKS[ci]
                t = xp.tile([128, NT, w], F8E3, tag="x5", name=f"x5_{ci}")
                o = L5_OFF[ci]
                nc.sync.dma_start(out=t[:], in_=fm_ap(x5_d)[:, :, o:o + w])
                x5_sb[ci] = t

            dma_leaf(0)
            dma_x5(0)
            idm_sb = const.tile([128, 128], F16, tag="idm", name="idmsb")
            nc.sync.dma_start(out=idm_sb[:], in_=idm_d.ap())
            dma_leaf(1)
            dma_x5(1)
            w2_sb = const.tile([128, 8, D], F16, tag="w2", name="w2sb")
            w2ap = w2_d.ap().rearrange("(kt p) e -> p kt e", p=128)
            for k in range(4):  # kt-pair slices keep 1KB contiguous runs
                nc.sync.dma_start(
                    out=w2_sb[:, 2 * k:2 * k + 2, :], in_=w2ap[:, 2 * k:2 * k + 2, :]
                )
            dma_leaf(2)
            dma_x5(2)
            dma_leaf(3)
            dma_x5(3)
            dma_leaf(4)
            dma_x5(4)
            dma_leaf(5)
            dma_x5(5)
            x4_sb = const.tile([128, NT, NLOC[4]], F8E3, tag="x4", name="x4sb")
            nc.sync.dma_start(out=x4_sb[:], in_=fm_ap(x4_d))
            xsm_sb = const.tile([128, NT, 175], F16, tag="xsm", name="xsmsb")
            nc.sync.dma_start(out=xsm_sb[:], in_=fm_ap(xsm_d))
            # loaded last: the DMA engines idle from ~31us and w4 is first
            # needed by the tail cascade at ~50us
            w4_sb = const.tile([128, 16, D], F16, tag="w4", name="w4sb")
            nc.sync.dma_start(
                out=w4_sb[:], in_=w4_d.ap().rearrange("(bk p) e -> p bk e", p=128)
            )

            # persistent per-level h tiles (feature-major, storage order)
            h5 = hp.tile([128, NT, NLOC[5]], F16, tag="h5", name="h5")
            h4 = hp.tile([128, NT, NLOC[4]], F16, tag="h4", name="h4")
            h3 = hp.tile([128, NT, NLOC[3]], F16, tag="h3", name="h3")
            h2 = hp.tile([128, NT, NLOC[2]], F16, tag="h2", name="h2")
            h1l = hp.tile([128, NT, NLOC[1]], F16, tag="h1l", name="h1l")
            # [p, r, t] layout (padded): the act writes a transposed view,
            # making the node-major cc_in write a single 3-dim DMA
            h0l = hp.tile([128, 2, 5], F16, tag="h0l", name="h0l")

            # ---------------- compute helpers ------------------------------
            def weighted_sums(Hb, w):
                """Ul = 3*H0 + 2*H1 + H2 ; Ur = H1 + 2*H2 + 3*H3 (f16).

                Hb(b) -> [128, NT, w] AP of child block b.
                ts runs 4x, tt 2x (f16); stt runs 1x but 4 ops for small w.
                """
                Ul = up.tile([128, NT, w], F16, tag="Ul", name="Ul")
                Ur = up.tile([128, NT, w], F16, tag="Ur", name="Ur")
                if w >= 128:
                    tA = tmpp.tile([128, NT, w], F16, tag="tA", name="tA")
                    tB = tmpp.tile([128, NT, w], F16, tag="tB", name="tB")
                    nc.vector.tensor_scalar_mul(tA[:], Hb(0), 3.0)
                    nc.vector.tensor_scalar_mul(tB[:], Hb(1), 2.0)
                    nc.vector.tensor_add(tA[:], tA[:], tB[:])
                    nc.vector.tensor_add(Ul[:], tA[:], Hb(2))
                    nc.vector.tensor_scalar_mul(tA[:], Hb(3), 3.0)
                    nc.vector.tensor_scalar_mul(tB[:], Hb(2), 2.0)
                    nc.vector.tensor_add(tA[:], tA[:], tB[:])
                    nc.vector.tensor_add(Ur[:], tA[:], Hb(1))
                else:
                    tA = tmpp.tile([128, NT, w], F16, tag="tA", name="tA")
                    tB = tmpp.tile([128, NT, w], F16, tag="tB", name="tB")
                    nc.vector.scalar_tensor_tensor(
                        out=tA[:], in0=Hb(0), scalar=3.0, in1=Hb(2),
                        op0=mult, op1=add,
                    )
                    nc.vector.scalar_tensor_tensor(
                        out=Ul[:], in0=Hb(1), scalar=2.0, in1=tA[:],
                        op0=mult, op1=add,
                    )
                    nc.vector.scalar_tensor_tensor(
                        out=tB[:], in0=Hb(3), scalar=3.0, in1=Hb(1),
                        op0=mult, op1=add,
                    )
                    nc.vector.scalar_tensor_tensor(
                        out=Ur[:], in0=Hb(2), scalar=2.0, in1=tB[:],
                        op0=mult, op1=add,
                    )
                return Ul, Ur

            def mm_unit(Ul, Ur, uoff, x_fn, h_out, hoff, w, pool_x=None):
                """agg[psum] = W2.T @ [Ul;Ur][uoff:uoff+w] + x ; tanh -> h_out.

                x added either by an identity matmul *after* the 8 W2 matmuls
                (so the group starts without waiting for x), or — when
                pool_x=(x_tile, xoff) — by a GpSimd PSUM add, freeing the PE.
                One activation per et-pair ([128,2,w] PSUM slab).
                """
                for p in range(2):
                    # one full PSUM bank per et (j) so each start/stop group
                    # owns its 2KB zero-region; Act reads both banks at once
                    ps = psum.tile([128, 2, 512], FP32, tag="ps", name="ps")
                    for j in range(2):
                        et = 2 * p + j
                        for kt in range(8):
                            nc.tensor.matmul(
                                ps[:, j, :w],
                                w2_sb[:, kt, et * 128:(et + 1) * 128],
                                (Ul if kt < 4 else Ur)[:, kt % 4, uoff:uoff + w],
                                start=(kt == 0),
                                stop=(pool_x is not None and kt == 7),
                            )
                        if pool_x is None:
                            nc.tensor.matmul(
                                ps[:, j, :w], idm_sb[:], x_fn(et),
                                start=False, stop=True,
                            )
                    if pool_x is not None:
                        xt, xo = pool_x
                        nc.gpsimd.tensor_add(
                            ps[:, :, :w], ps[:, :, :w],
                            xt[:, 2 * p:2 * p + 2, xo:xo + w],
                        )
                    nc.scalar.activation(
                        out=h_out[:, 2 * p:2 * p + 2, hoff:hoff + w],
                        in_=ps[:, :, :w], func=Tanh,
                    )

            # ---------------- schedule -------------------------------------
            def weighted_sums_e3(Hb, w):
                """U-build over fp8e3m4 children: convert each block to f16
                once (ts runs 2x on 1-byte dtypes), then pure-f16 ts/tt."""
                Ul = up.tile([128, NT, w], F16, tag="Ul", name="Ul")
                Ur = up.tile([128, NT, w], F16, tag="Ur", name="Ur")
                cA = tmpp.tile([128, NT, w], F16, tag="cA", name="cA")
                cB = tmpp.tile([128, NT, w], F16, tag="cB", name="cB")
                cC = tmpp.tile([128, NT, w], F16, tag="cC", name="cC")
                cD = tmpp.tile([128, NT, w], F16, tag="cD", name="cD")
                t1 = tmpp.tile([128, NT, w], F16, tag="tA", name="t1")
                t2 = tmpp.tile([128, NT, w], F16, tag="tB", name="t2")
                nc.vector.tensor_scalar_mul(cA[:], Hb(0), 3.0)
                nc.vector.tensor_scalar_mul(cB[:], Hb(1), 1.0)
                nc.vector.tensor_scalar_mul(cC[:], Hb(2), 1.0)
                nc.vector.tensor_scalar_mul(cD[:], Hb(3), 3.0)
                nc.vector.tensor_scalar_mul(t1[:], cB[:], 2.0)
                nc.vector.tensor_add(t1[:], cA[:], t1[:])
                nc.vector.tensor_add(Ul[:], t1[:], cC[:])
                nc.vector.tensor_scalar_mul(t2[:], cC[:], 2.0)
                nc.vector.tensor_add(t2[:], cD[:], t2[:])
                nc.vector.tensor_add(Ur[:], t2[:], cB[:])
                return Ul, Ur

            def weighted_sums_e3(Hb, w):
                """U-build over fp8e3m4 children.

                Each block crosses to f16 once: cA=3*H0, cD=3*H3 on Act
                (scaled copies), cB=H1 on Pool, cC=H2 on DVE; the remaining
                adds run in f16 DVE 4x/2x modes."""
                Ul = up.tile([128, NT, w], F16, tag="Ul", name="Ul")
                Ur = up.tile([128, NT, w], F16, tag="Ur", name="Ur")
                cA = tmpp.tile([128, NT, w], F16, tag="cA", name="cA")
                cB = tmpp.tile([128, NT, w], F16, tag="cB", name="cB")
                cC = tmpp.tile([128, NT, w], F16, tag="cC", name="cC")
                cD = tmpp.tile([128, NT, w], F16, tag="cD", name="cD")
                t1 = tmpp.tile([128, NT, w], F16, tag="tA", name="t1")
                t2 = tmpp.tile([128, NT, w], F16, tag="tB", name="t2")
                nc.gpsimd.tensor_scalar_mul(cB[:], Hb(1), 1.0)
                nc.scalar.mul(cA[:], Hb(0), 3.0)
                nc.scalar.mul(cD[:], Hb(3), 3.0)
                nc.vector.tensor_scalar_mul(cC[:], Hb(2), 1.0)
                nc.vector.tensor_scalar_mul(t1[:], cB[:], 2.0)
                nc.vector.tensor_add(t1[:], cA[:], t1[:])
                nc.vector.tensor_add(Ul[:], t1[:], cC[:])
                nc.vector.tensor_scalar_mul(t2[:], cC[:], 2.0)
                nc.vector.tensor_add(t2[:], cD[:], t2[:])
                nc.vector.tensor_add(Ur[:], t2[:], cB[:])
                return Ul, Ur

            U5 = {}

            def do_l5_chunk(ci, phase="UM"):
                lt, xt = leaf_sb[ci], x5_sb[ci]
                cw = L5_CHUNKS[ci]
                # per-256-slice U-build + matmuls: short pipeline depth
                # (leaf -> convs -> adds -> mm), chunk's own convs sit ahead
                # of its tanhs on the Act queue
                for u0 in range(0, cw, UNIT):
                    w = min(UNIT, cw - u0)
                    if "U" in phase:
                        U5[(ci, u0)] = weighted_sums_e3(
                            lambda b, u0=u0, w=w, cw=cw, lt=lt:
                                lt[:, :, b * cw + u0:b * cw + u0 + w],
                            w,
                        )
                    if "M" in phase:
                        Ul, Ur = U5[(ci, u0)]
                        mm_unit(
                            Ul, Ur, 0,
                            lambda et, u0=u0, w=w, xt=xt: xt[:, et, u0:u0 + w],
                            h5, L5_OFF[ci] + u0, w,
                        )

            def do_sub(child_h, gcoff, blk, soff, x_sb, xoff, h_out, hoff, w):
                """Sub-chunk reading child group at gcoff (4 blocks of blk),
                columns [soff, soff+w) of each block."""
                Ul, Ur = weighted_sums(
                    lambda b: child_h[
                        :, :, gcoff + b * blk + soff:gcoff + b * blk + soff + w
                    ],
                    w,
                )
                mm_unit(
                    Ul, Ur, 0,
                    lambda et: x_sb[:, et, xoff:xoff + w],
                    h_out, hoff, w,
                )

            def mm_unit_b(Hb_fn, x_fn, h_out, hoff, w, out_ap=None):
                """Tail unit via the 4-child form: agg = sum_b h_b @ Vb.T + x.

                Skips the DVE U-build (shorter act->mm->act chain); 2x PE
                work but w<=32 makes that negligible.  Hb_fn(b, kt) -> the
                [128, w] moving slice of child block b, d-tile kt.
                """
                for p in range(2):
                    ps = psum.tile([128, 2, 512], FP32, tag="ps", name="ps")
                    for j in range(2):
                        et = 2 * p + j
                        for b in range(B):
                            for kt in range(4):
                                nc.tensor.matmul(
                                    ps[:, j, :w],
                                    w4_sb[:, 4 * b + kt, et * 128:(et + 1) * 128],
                                    Hb_fn(b, kt),
                                    start=(b == 0 and kt == 0), stop=False,
                                )
                        nc.tensor.matmul(
                            ps[:, j, :w], idm_sb[:], x_fn(et),
                            start=False, stop=True,
                        )
                    nc.scalar.activation(
                        out=(
                            h_out[:, 2 * p:2 * p + 2, hoff:hoff + w]
                            if out_ap is None else out_ap[p]
                        ),
                        in_=ps[:, :, :w], func=Tanh,
                    )

            def do_sub_b(child_h, coff, x_sb, xoff, h_out, hoff, w, out_ap=None):
                mm_unit_b(
                    lambda b, kt: child_h[:, kt, coff + b * w:coff + (b + 1) * w],
                    lambda et: x_sb[:, et, xoff:xoff + w],
                    h_out, hoff, w, out_ap=out_ap,
                )

            # in-order engines: emit each item only after its producer's act
            # is >=2 slots earlier.  L4 sub t <- h5 group t; the last group
            # is computed in two 64-col pieces to shorten the trailing chain.
            do_l5_chunk(0)
            do_l5_chunk(1)
            do_l5_chunk(2)
            do_l5_chunk(3)
            do_sub(h5, 0, 128, 0, x4_sb, 0, h4, 0, 128)        # s0 <- c0..c2
            do_l5_chunk(4)
            do_sub(h5, 512, 128, 0, x4_sb, 128, h4, 128, 128)  # s1 <- c3
            do_l5_chunk(5)
            do_sub(h5, 1024, 128, 0, x4_sb, 256, h4, 256, 128)  # s2 <- c4
            do_sub(h4, 0, 32, 0, xsm_sb, 0, h3, 0, 32)         # u0 <- s0
            do_sub(h5, 1536, 128, 0, x4_sb, 384, h4, 384, 64)  # s3a <- c5
            do_sub(h5, 1536, 128, 64, x4_sb, 448, h4, 448, 64)  # s3b <- c5
            do_sub(h4, 128, 32, 0, xsm_sb, 32, h3, 32, 32)     # u1 <- s1
            do_sub(h4, 256, 32, 0, xsm_sb, 64, h3, 64, 32)     # u2 <- s2
            do_sub_b(h4, 384, xsm_sb, 96, h3, 96, 32)   # u3 <- s3a+s3b (chain)
            do_sub_b(h3, 0, xsm_sb, 128, h2, 0, 32)     # L2 <- h3[0:128]
            do_sub_b(h2, 0, xsm_sb, 160, h1l, 0, 8)     # L1 <- h2[0:32]
            do_sub_b(                                   # L0 <- h1l[0:8]
                h1l, 0, xsm_sb, 168, h0l, 0, 2,
                out_ap=[
                    h0l[:, :, 0:2].rearrange("p r t -> p t r"),
                    h0l[:, :, 2:4].rearrange("p r t -> p t r"),
                ],
            )

            # ---- tail: AllGather the 16 level-2 hiddens, redundant 1+0 ----
            # node-major [2, D] rows (feature-contiguous) so the gathered
            # [16, D] reassembles into feature-major SBUF in ONE 3-dim DMA;
            # cc_in itself needs one DMA per row
            # rows use p-major feature order (f = 4p+t) so both the write and
            # the gather reassembly keep a contiguous innermost dim (t)
            cc_in = dram.tile([2, D], F16, tag="cc_in", name="cc_in")
            cc_out = dram.tile([NCORES * 2, D], F16, tag="cc_out", name="cc_out")
            nc.sync.dma_start(
                out=cc_in[:, :].rearrange("r (p t) -> p r t", p=128),
                in_=h0l[:, :, 0:4],
            )
            if with_tail:
                nc.gpsimd.collective_compute(
                    "AllGather",
                    mybir.AluOpType.bypass,
                    replica_groups=[list(range(NCORES))],
                    ins=[cc_in.opt()],
                    outs=[cc_out.opt()],
                )
                h2g_src = cc_out[:, :]
                nk = 16
            else:
                # collective-free cost-sim variant: the +10us allgather
                # allowance accounts for the collective; read cc_in directly
                h2g_src = cc_in[:, :]
                nk = 2
            # row k = 2c+r holds global level-2 node inv2[k]; children of
            # level-1 node p are rows 4b+p -> Hb(b) = rows [4b, 4b+4).
            # SBUF layout [p, k, t] (t innermost, padded to 5) -> one 3-dim
            # DMA with contiguous last dim on both sides; consumers read a
            # transposed view.
            h2g_fm = const.tile([128, 16, 5], F16, tag="h2gfm", name="h2gfm")
            nc.sync.dma_start(
                out=h2g_fm[:, 0:nk, 0:4],
                in_=h2g_src.rearrange("k (p t) -> p k t", p=128),
            )

            # level 1 (4 nodes): children of node p are h2g rows 4p..4p+3
            h1_fm = const.tile([128, NT, 4], F16, tag="h1fm", name="h1fm")
            mm_unit_b(
                lambda b, kt: h2g_fm[:, 4 * b:4 * b + 4, kt],
                lambda et: xsm_sb[:, et, 170:174],
                h1_fm, 0, 4,
            )

            # root: just another 1-column feature-major unit (fp32 out)
            h0_fm = const.tile([128, NT, 1], FP32, tag="h0fm", name="h0fm")
            mm_unit_b(
                lambda b, kt: h1_fm[:, kt, b:b + 1],
                lambda et: xsm_sb[:, et, 174:175],
                h0_fm, 0, 1,
            )
            nc.sync.dma_start(
                out=out_d.ap().rearrange("o (t p) -> p t o", p=128),
                in_=h0_fm[:],
            )

    nc.compile()
    return nc


_NC_CACHE = {}


def _get_nc():
    if "nc" not in _NC_CACHE:
        _NC_CACHE["nc"] = _build_nc()
    return _NC_CACHE["nc"]


# ---------------------------------------------------------------- entry point
def kernel(vectors, wl, wr, branching, depth):
    out, _ = _run(vectors, wl, wr, branching, depth, trace=False)
    return out


def _run(vectors, wl, wr, branching, depth, trace=False):
    assert int(branching) == B and int(depth) == DEPTH
    import time as _time

    in_maps = _make_in_maps(vectors, wl, wr)
    nc = _get_nc()
    last = None
    for attempt in range(6):
        try:
            res = run_bass_kernel_spmd(
                nc, in_maps, core_ids=list(range(NCORES)), trace=trace
            )
            break
        except Exception as e:
            # transient device errors clear after a reset cycle
            last = e
            _time.sleep(5.0 * (attempt + 1))
    else:
        raise last
    return np.asarray(res.results[0]["out"], dtype=np.float32), res


def _make_in_maps(vectors, wl, wr):
    import ml_dtypes

    vectors = np.asarray(vectors, dtype=np.float32)
    off = [(B**l - 1) // (B - 1) for l in range(DEPTH + 1)]

    def fm(rows, dt=F16np):
        return np.ascontiguousarray(rows.T.astype(dt))

    base = {}
    # W2 = [wl.T ; wr.T] / 3 : agg = W2.T @ [u_l' ; u_r']
    w2 = np.concatenate([np.asarray(wl).T, np.asarray(wr).T], axis=0) / 3.0
    base["w2"] = np.ascontiguousarray(w2, dtype=F16np)
    # W4 = stack_b (lc_b*wl + rc_b*wr).T : agg = sum_b W4_b.T @ h_b
    wl32, wr32 = np.asarray(wl, np.float32), np.asarray(wr, np.float32)
    lc = np.array([3, 2, 1, 0], np.float32) / 3
    rc = np.array([0, 1, 2, 3], np.float32) / 3
    w4 = np.concatenate(
        [(lc[b] * wl32 + rc[b] * wr32).T for b in range(B)], axis=0
    )
    base["w4"] = np.ascontiguousarray(w4, dtype=F16np)
    base["identm"] = np.eye(128, dtype=F16np)

    # core c owns the two global level-2 subtrees with storage positions
    # {2c, 2c+1}: pos = 4*b + p for global level-2 node j = 4p + b.
    g2 = np.arange(16, dtype=np.int64)
    pos = 4 * (g2 % 4) + (g2 // 4)
    inv2 = np.empty(16, dtype=np.int64)
    inv2[pos] = g2

    xt1 = vectors[off[1]:off[1] + 4]
    xt0 = vectors[off[0]:off[0] + 1]

    in_maps = []
    for c in range(NCORES):
        roots = inv2[2 * c:2 * c + 2]
        m = dict(base)

        def grows(l):
            stor = _P[l]
            r = stor >> (2 * l)
            q = stor & ((1 << (2 * l)) - 1)
            return off[l + 2] + roots[r] * (4**l) + q

        m["leaf"] = fm(vectors[grows(6)], ml_dtypes.float8_e3m4)
        m["x5"] = fm(vectors[grows(5)], ml_dtypes.float8_e3m4)
        m["x4"] = fm(vectors[grows(4)], ml_dtypes.float8_e3m4)
        xsm = np.concatenate(
            [vectors[grows(3)], vectors[grows(2)], vectors[grows(1)],
             vectors[grows(0)], xt1, xt0], axis=0
        )
        m["xsm"] = fm(xsm)
        in_maps.append(m)
    return in_maps


if __name__ == "__main__":
    sys.path.insert(0, "/root/problem")
    d = np.load("/root/problem/ref_cache.npz")
    out = kernel(d["vectors"], d["wl"], d["wr"], 4, 8)
    exp = d["expected"]
    rel = np.linalg.norm(out - exp) / np.linalg.norm(exp)
    print("out[0,:5]:", out[0, :5])
    print("rel:", rel, "absmax:", np.abs(out - exp).max())
